# revision 32
# baseline (speedup 1.0000x reference)
"""ARMA GNN (nn_ARMA_85976655332070) Trainium2 Bass kernel, 8 NeuronCores.

Strategy v3 (graph/data parallel per sharding hint, optimized):
 - Nodes sharded contiguously across 8 cores (12500/core, padded to 12544).
 - gcn_norm folded into per-edge weights on host (structure-only float
   preprocessing); no deg/dis pass on device.
 - Linearity trick: segment_sum commutes with the per-stack feature
   transforms, so each phase communicates the UNTRANSFORMED node table
   (width H for t=0 phases, K*H for t=1) and applies init_w / w AFTER
   aggregation, per dst block. Halves t=0 traffic and removes the P0 pass.
 - All tables / messages / one-hot tiles in bf16 (halves DMA + AllGather
   bytes); PSUM accumulation stays f32; dense weights bf16.
 - Edges grouped by (dst-block, src-quarter); per sparse phase the work is
   ordered half -> quarter-pair -> dst-group(4 blocks) -> quarter, with
   PSUM accumulation across the quarter pair and bf16 SBUF accumulators
   across pairs, so the in-order Pool engine never stalls behind the last
   AllGather and evictions stream early (launching next-phase AllGathers).

 v3 perf notes (measured on HW, 9.65ms -> ~5.0ms):
 - dma_gather is DESCRIPTOR-RATE bound (~8.25ns/desc on one SWDGE queue,
   independent of elem width or core count). Spreading gather calls
   round-robin over 4 SWDGE queues (num_swdge_queues=4) runs them at
   ~2.2-3ns/desc; gathers-only drops 8.3ms -> 4.5ms. single_packet=True
   hard-wedges the device (NRT_EXEC_UNIT_UNRECOVERABLE) - never use.
 - AllGathers are nearly free (~0.35ms total for all 16); do not optimize.
 - One-hot m tiles are built per gather call in 2 big DVE tensor_tensor
   ops (iota/dl/norm broadcast_to views) instead of per-tile tensor_scalar
   (20x fewer DVE instructions).
 - Aggregation psums are packed 4 (t=0) / 2 (t=1) dst blocks per PSUM
   bank. PSUM rule: start=True clears has_written for the WHOLE bank, so
   only the first matmul touching a bank sets start; later blocks' first
   writes rely on per-element overwrite-where-unset.
 - Cross-pair accumulator acc_sb is bf16 (frees 26KB/partition -> gather
   pool 6 bufs deep, m pool 5) at ~+0.04% rel err (0.0054 -> 0.0057).
"""

import math
import sys
from dataclasses import dataclass

import numpy as np
import ml_dtypes

sys.path.insert(0, "/opt/trn_rl_repo")

import bass_rust  # noqa: E402
import concourse.bass as bass  # noqa: E402
import concourse.mybir as mybir  # noqa: E402
import concourse.tile as tile  # noqa: E402
from concourse import library_config  # noqa: E402
from concourse.library_overlay import lower_extended_insts  # noqa: E402

F32 = mybir.dt.float32
F32R = mybir.dt.float32r
BF16 = mybir.dt.bfloat16
F8 = mybir.dt.float8e4
I16 = mybir.dt.int16
ALU = mybir.AluOpType
ACT = mybir.ActivationFunctionType
BFNP = ml_dtypes.bfloat16


def fix_excess_waits(nc, limit=1):
    """This walrus build allows very few sync-waits per instruction; move the
    excess onto single-wait EventSemaphore carriers placed just before."""
    n = 0
    for f in nc.m.functions:
        for bb in f.blocks:
            out = []
            for inst in bb.instructions:
                si = inst.sync_info
                if si is not None and len(si.on_wait) > limit:
                    waits = list(si.on_wait)
                    keep = [] if type(inst).__name__ == "InstDrain" else waits[:limit]
                    for w in waits[len(keep):]:
                        ev = bass_rust.InstEventSemaphore(
                            name=f"wsplit_{n}", engine=inst.engine, ins=[], outs=[]
                        )
                        n += 1
                        ev.sync_info = bass_rust.SyncInfo(on_wait=[w], on_update=[])
                        out.append(ev)
                    inst.sync_info = bass_rust.SyncInfo(
                        on_wait=keep, on_update=list(si.on_update)
                    )
                out.append(inst)
            bb.instructions = out
    return n


@dataclass(frozen=True)
class Cfg:
    N: int = 100_000
    F: int = 512
    H: int = 128
    K: int = 2
    T: int = 2
    L: int = 2
    C: int = 40
    W: int = 8  # cores
    GB: int = 8  # dst blocks per matmul group


@dataclass(frozen=True)
class Derived:
    NPper: int
    NB: int
    NPAD: int
    QBLK: int
    qstart: tuple
    qrows: tuple
    groups: tuple
    halves: tuple
    FB: int
    KH: int


def derive(cfg: Cfg) -> Derived:
    assert cfg.N % cfg.W == 0 and cfg.H == 128 and cfg.F % 128 == 0
    NPper = cfg.N // cfg.W
    NB = (NPper + 127) // 128
    NPAD = NB * 128
    QBLK = (NB + 3) // 4
    qstart = tuple(min(i * QBLK, NB) for i in range(5))
    qrows = tuple((qstart[i + 1] - qstart[i]) * 128 for i in range(4))
    assert all(cfg.W * qr <= 32767 for qr in qrows), "int16 gather idx overflow"
    groups = tuple(tuple(range(i, min(i + cfg.GB, NB)))
                   for i in range(0, NB, cfg.GB))
    NG = len(groups)
    halves = (tuple(range(0, (NG + 1) // 2)), tuple(range((NG + 1) // 2, NG)))
    return Derived(NPper, NB, NPAD, QBLK, qstart, qrows, groups, halves,
                   cfg.F // 128, cfg.K * cfg.H)


def call_layout(cfg, dv, TGC):
    """Static per-core call order (half, pair, group, quarter)."""
    GT = TGC * 128
    calls = []
    slot = 0
    for half in dv.halves:
        for pair in range(2):
            for g in half:
                for q in (2 * pair, 2 * pair + 1):
                    blocks = dv.groups[g]
                    calls.append((g, q, blocks, slot, slot // 128))
                    slot += len(blocks) * GT
    return calls, slot


def pack_host(cfg, dv, x, edge_index, edge_weight, Win, b_in, init_w, w,
              root_w, bias, Wout, b_out):
    """Host preprocessing: gcn_norm on edge weights (float32), index packing,
    padding/transposes, bf16 weight conversion."""
    W, NPper, NB, NPAD, GB = cfg.W, dv.NPper, dv.NB, dv.NPAD, cfg.GB
    src = np.asarray(edge_index[0], dtype=np.int64)
    dst = np.asarray(edge_index[1], dtype=np.int64)
    ew = np.asarray(edge_weight, dtype=np.float32)
    E = src.shape[0]

    deg = np.zeros(cfg.N, np.float32)
    np.add.at(deg, dst, ew)
    dis = np.where(deg > 0, 1.0 / np.sqrt(np.maximum(deg, 1e-30)), 0.0)
    dis = dis.astype(np.float32)
    norm = (dis[src] * ew * dis[dst]).astype(np.float32)

    c = dst // NPper
    dloc = dst - c * NPper
    b = dloc >> 7
    dl = (dloc & 127).astype(np.float32)
    rs = src // NPper
    ns = src - rs * NPper
    bs = ns >> 7
    qe = np.minimum(bs // dv.QBLK, 3)
    qs = np.asarray(dv.qstart)[qe] * 128
    row = rs * np.asarray(dv.qrows)[qe] + (ns - qs)

    counts = np.zeros((W, NB, 4), np.int64)
    np.add.at(counts, (c, b, qe), 1)
    TGC = max(1, int(math.ceil(counts.max() / 128.0)))
    GT = TGC * 128

    calls, SLOTS = call_layout(cfg, dv, TGC)
    NTILES = SLOTS // 128
    NG = len(dv.groups)
    cb_arr = np.zeros((NG, 4), np.int64)
    for (g, q, blocks, slot_base, tile_base) in calls:
        cb_arr[g, q] = slot_base

    g_of_b = np.arange(NB) // GB
    j_of_b = np.arange(NB) % GB

    key = (c * NB + b) * 4 + qe
    counts_flat = np.zeros(W * NB * 4, np.int64)
    np.add.at(counts_flat, key, 1)
    starts = np.zeros_like(counts_flat)
    starts[1:] = np.cumsum(counts_flat)[:-1]
    order = np.argsort(key, kind="stable")
    rank = np.empty(E, np.int64)
    rank[order] = np.arange(E) - starts[key[order]]

    slot_lin = cb_arr[g_of_b[b], qe] + j_of_b[b] * GT + rank

    gidx16 = np.zeros((W, 16, SLOTS // 16), np.int16)
    gidx16[c, slot_lin % 16, slot_lin // 16] = row.astype(np.int16)
    gidx = np.tile(gidx16, (1, 8, 1))

    tc_edge = (cb_arr[g_of_b[b], qe] >> 7) + j_of_b[b] * TGC + (rank >> 7)
    p_edge = rank & 127
    gdl = np.zeros((W, 128, NTILES), BFNP)
    gdl[c, p_edge, tc_edge] = dl
    gnw = np.zeros((W, 128, NTILES), np.float32)
    gnw[c, p_edge, tc_edge] = norm
    gnw = gnw.astype(BFNP)
    
    # x: pad + per-core transpose -> [W, FB, 128, NPAD] (f32)
    x = np.asarray(x, dtype=np.float32)
    xpad = np.zeros((W, NPAD, cfg.F), np.float32)
    xpad[:, :NPper, :] = x.reshape(W, NPper, cfg.F)
    xT4 = np.ascontiguousarray(
        xpad.reshape(W, NPAD, dv.FB, 128).transpose(0, 2, 3, 1))

    K, H = cfg.K, cfg.H
    win4 = np.ascontiguousarray(
        np.asarray(Win, np.float32).reshape(dv.FB, 128, H))
    initw2 = np.ascontiguousarray(
        np.asarray(init_w, np.float32).transpose(0, 2, 1, 3)
        .reshape(cfg.L, H, K * H)).astype(BFNP)
    rootw2 = np.ascontiguousarray(
        np.asarray(root_w, np.float32).transpose(0, 1, 3, 2, 4)
        .reshape(cfg.L, cfg.T, H, K * H)).astype(BFNP)
    bias2 = np.ascontiguousarray(
        np.asarray(bias, np.float32).reshape(cfg.L, cfg.T, 1, K * H)
    ).astype(BFNP)
    wmat = np.ascontiguousarray(np.asarray(w, np.float32)).astype(BFNP)
    wout = np.asarray(Wout, np.float32).astype(BFNP)
    bout = np.asarray(b_out, np.float32).reshape(1, cfg.C).astype(BFNP)
    b_in_col = np.asarray(b_in, np.float32).reshape(H, 1)
    iota = np.tile(np.arange(128, dtype=np.float32), (128, 1)).astype(BFNP)

    per_core = []
    for ci in range(W):
        per_core.append(dict(
            xT4=xT4[ci], gidx=gidx[ci], gdl=gdl[ci], gnw=gnw[ci],
            win4=win4, b_in_col=b_in_col, initw2=initw2, rootw2=rootw2,
            bias2=bias2, wmat=wmat, wout=wout, bout=bout, iota=iota,
        ))
    return per_core, TGC


def build_nc(cfg: Cfg, dv: Derived, TGC: int, repeat: int = 1,
             debug: bool = False, no_collectives: bool = False,
             no_gathers: bool = False):
    from concourse.masks import make_identity

    K, H, T, L, C = cfg.K, cfg.H, cfg.T, cfg.L, cfg.C
    KH, NB, NPAD, FB, GB = dv.KH, dv.NB, dv.NPAD, dv.FB, cfg.GB
    GT = TGC * 128
    calls, SLOTS = call_layout(cfg, dv, TGC)
    NTILES = SLOTS // 128
    MAXHALF = max(sum(len(dv.groups[g]) for g in half) for half in dv.halves)
    half_base = [min(min(dv.groups[g]) for g in half) for half in dv.halves]

    nc = bass.Bass(num_swdge_queues=4)
    # ---- params
    xT4 = nc.declare_dram_parameter("xT4", [FB, 128, NPAD], F32R, isOutput=False)
    gidx = nc.declare_dram_parameter("gidx", [128, SLOTS // 16], I16,
                                     isOutput=False)
    gdl = nc.declare_dram_parameter("gdl", [128, NTILES], BF16,
                                    isOutput=False)
    gnw = nc.declare_dram_parameter("gnw", [128, NTILES], BF16,
                                    isOutput=False)
    win4 = nc.declare_dram_parameter("win4", [FB, 128, H], F32R, isOutput=False)
    b_in_col = nc.declare_dram_parameter("b_in_col", [H, 1], F32,
                                         isOutput=False)
    initw2 = nc.declare_dram_parameter("initw2", [L, H, KH], BF16,
                                       isOutput=False)
    rootw2 = nc.declare_dram_parameter("rootw2", [L, T, H, KH], BF16,
                                       isOutput=False)
    bias2 = nc.declare_dram_parameter("bias2", [L, T, 1, KH], BF16,
                                      isOutput=False)
    wmat = nc.declare_dram_parameter("wmat", [L, max(1, T - 1), K, H, H], BF16,
                                     isOutput=False)
    wout = nc.declare_dram_parameter("wout", [H, C], BF16, isOutput=False)
    bout = nc.declare_dram_parameter("bout", [1, C], BF16, isOutput=False)
    iota_in = nc.declare_dram_parameter("iota", [128, 128], BF16,
                                        isOutput=False)
    logits = nc.declare_dram_parameter("logits", [NPAD, C], F32, isOutput=True)
    if debug:
        dbg_h = nc.declare_dram_parameter("dbg_h", [NPAD, H], BF16,
                                          isOutput=True)
        dbg_agg = nc.declare_dram_parameter("dbg_agg", [NPAD, H], F32,
                                            isOutput=True)
        dbg_out0 = nc.declare_dram_parameter("dbg_out0", [NPAD, KH], BF16,
                                             isOutput=True)

    # ---- internal DRAM: per-quarter z (width H) and y (width KH) tables
    zin_q, ztab_q, yin_q, ytab_q = [], [], [], []
    for q in range(4):
        r = dv.qrows[q]
        zin_q.append(nc.dram_tensor(f"zin_{q}", [r, H], BF16))
        ztab_q.append(nc.dram_tensor(f"ztab_{q}", [cfg.W * r, H], BF16,
                                     addr_space="Shared"))
        yin_q.append(nc.dram_tensor(f"yin_{q}", [r, KH], BF16))
        ytab_q.append(nc.dram_tensor(f"ytab_{q}", [cfg.W * r, KH], BF16,
                                     addr_space="Shared"))

    rg = [list(range(cfg.W))]

    def qb_of(b):
        q = min(b // dv.QBLK, 3)
        return q, (b - dv.qstart[q]) * 128

    with tile.TileContext(nc) as tc:
        nc.gpsimd.load_library(library_config.mlp)
        import contextlib
        ctx = contextlib.ExitStack()
        with ctx:
            cpool = ctx.enter_context(tc.tile_pool(name="const", bufs=1))
            xpool = ctx.enter_context(tc.tile_pool(name="xin", bufs=3))
            gpool = ctx.enter_context(tc.tile_pool(name="gath", bufs=3))
            mpool = ctx.enter_context(tc.tile_pool(name="mbuild", bufs=3))
            ypool = ctx.enter_context(tc.tile_pool(name="ywrite", bufs=3))
            epool = ctx.enter_context(tc.tile_pool(name="evict", bufs=8))
            pp_agg = ctx.enter_context(
                tc.tile_pool(name="pagg", bufs=5, space="PSUM"))
            pp_root = ctx.enter_context(
                tc.tile_pool(name="proot", bufs=2, space="PSUM"))
            pp_tr = ctx.enter_context(
                tc.tile_pool(name="ptr", bufs=1, space="PSUM"))

            # ---- persistent SBUF
            iota_sb = cpool.tile([128, 128], BF16, tag="iota")
            nc.sync.dma_start(out=iota_sb[:], in_=iota_in[:])
            ident_raw = cpool.tile([128, 128], F32, tag="ident_raw")
            make_identity(nc, ident_raw[:])
            ident_f = cpool.tile([128, 128], F32R, tag="ident_f")
            nc.vector.tensor_copy(out=ident_f[:], in_=ident_raw[:])
            ident_b = cpool.tile([128, 128], BF16, tag="ident_b")
            nc.vector.tensor_copy(out=ident_b[:], in_=ident_raw[:])
            ones_row = cpool.tile([1, 128], BF16, tag="ones_row")
            nc.gpsimd.memset(ones_row[:], 1.0)
            binc = cpool.tile([H, 1], F32, tag="binc")
            nc.sync.dma_start(out=binc[:], in_=b_in_col[:])
            win_sb = cpool.tile([128, FB * H], F32R, tag="win")
            for fb in range(FB):
                nc.sync.dma_start(out=win_sb[:, fb * H:(fb + 1) * H],
                                  in_=win4[fb])
            initw_sb = cpool.tile([128, L * KH], BF16, tag="initw")
            for l in range(L):
                nc.sync.dma_start(out=initw_sb[:, l * KH:(l + 1) * KH],
                                  in_=initw2[l])
            rootw_sb = cpool.tile([128, L * T * KH], BF16, tag="rootw")
            for l in range(L):
                for t in range(T):
                    o = (l * T + t) * KH
                    nc.sync.dma_start(out=rootw_sb[:, o:o + KH],
                                      in_=rootw2[l, t])
            bias_sb = cpool.tile([1, L * T * KH], BF16, tag="bias")
            for l in range(L):
                for t in range(T):
                    o = (l * T + t) * KH
                    nc.sync.dma_start(out=bias_sb[:, o:o + KH], in_=bias2[l, t])
            wmat_sb = cpool.tile([128, L * max(1, T - 1) * K * H], BF16,
                                 tag="wmat")
            for l in range(L):
                for t in range(max(1, T - 1)):
                    for k in range(K):
                        o = ((l * max(1, T - 1) + t) * K + k) * H
                        nc.sync.dma_start(out=wmat_sb[:, o:o + H],
                                          in_=wmat[l, t, k])
            wout_sb = cpool.tile([H, C], BF16, tag="wout")
            nc.sync.dma_start(out=wout_sb[:], in_=wout[:])
            bout_sb = cpool.tile([1, C], BF16, tag="bout")
            nc.sync.dma_start(out=bout_sb[:], in_=bout[:])
            gidx_sb = cpool.tile([128, SLOTS // 16], I16, tag="gidx")
            nc.sync.dma_start(out=gidx_sb[:], in_=gidx[:])
            gdl_sb = cpool.tile([128, NTILES], BF16, tag="gdl")
            nc.sync.dma_start(out=gdl_sb[:], in_=gdl[:])
            gnw_sb = cpool.tile([128, NTILES], BF16, tag="gnw")
            nc.sync.dma_start(out=gnw_sb[:], in_=gnw[:])
            hT_sb = cpool.tile([128, NPAD], BF16, tag="hT")
            acc_sb = cpool.tile([128, MAXHALF * KH], BF16, tag="acc")

            _nidx_regs = {}

            def nidx_reg(v):
                if v not in _nidx_regs:
                    _nidx_regs[v] = nc.gpsimd.to_reg(v)
                return _nidx_regs[v]

            def m_bulk(tile_base, nt):
                """One-hot-times-norm tiles for a whole call in 2 DVE ops."""
                mt = mpool.tile([128, cfg.GB * TGC, 128], BF16, tag="m")
                msl = mt[:, :nt, :]
                iota_b = iota_sb[:].unsqueeze(1).broadcast_to([128, nt, 128])
                dl_b = gdl_sb[:, tile_base:tile_base + nt].unsqueeze(2) \
                    .broadcast_to([128, nt, 128])
                nw_b = gnw_sb[:, tile_base:tile_base + nt].unsqueeze(2) \
                    .broadcast_to([128, nt, 128])
                nc.vector.tensor_tensor(out=msl, in0=iota_b, in1=dl_b,
                                        op=ALU.is_equal)
                nc.vector.tensor_tensor(out=msl, in0=msl, in1=nw_b,
                                        op=ALU.mult)
                return mt

            call_info = {(g, q): (blocks, slot_base, tile_base)
                         for (g, q, blocks, slot_base, tile_base) in calls}
            gq_counter = [0]

            for _rep in range(repeat):
                def maybe_ag(b, p):
                    # launch quarter AllGather for phase p once its last
                    # block's table rows have been written
                    if p >= L * T:
                        return
                    for q in range(4):
                        if b == dv.qstart[q + 1] - 1:
                            if p % 2 == 0:
                                ins, outs = zin_q[q], ztab_q[q]
                            else:
                                ins, outs = yin_q[q], ytab_q[q]
                            if not no_collectives:
                                nc.gpsimd.collective_compute(
                                    "AllGather", ALU.bypass, replica_groups=rg,
                                    ins=[ins[:]], outs=[outs[:]])

                # ========== input: hT = relu(Win^T xT + b_in); z0 = h ======
                for g0 in range(0, NPAD, 256):
                    gs = min(256, NPAD - g0)
                    ps = pp_root.tile([128, KH], F32, tag="root")
                    for fb in range(FB):
                        xt = xpool.tile([128, 256], F32R, tag="xin")
                        nc.sync.dma_start(out=xt[:, :gs],
                                          in_=xT4[fb, :, g0:g0 + gs])
                        nc.tensor.matmul(ps[:, :gs],
                                         win_sb[:, fb * H:(fb + 1) * H],
                                         xt[:, :gs],
                                         start=(fb == 0), stop=(fb == FB - 1))
                    nc.scalar.activation(hT_sb[:, g0:g0 + gs], ps[:, :gs],
                                         ACT.Relu, bias=binc[:])
                    for b in range(g0 // 128, (g0 + gs) // 128):
                        bsl = slice(b * 128, (b + 1) * 128)
                        trp = pp_tr.tile([128, 128], BF16, tag="tr")
                        nc.tensor.transpose(trp[:], hT_sb[:, bsl], ident_b[:])
                        zt = ypool.tile([128, KH], BF16, tag="y")
                        nc.vector.tensor_copy(out=zt[:, :H], in_=trp[:])
                        q, r0 = qb_of(b)
                        nc.sync.dma_start(out=zin_q[q][r0:r0 + 128, :],
                                          in_=zt[:, :H])
                        if debug and _rep == 0:
                            nc.sync.dma_start(
                                out=dbg_h[b * 128:(b + 1) * 128, :],
                                in_=zt[:, :H])
                        maybe_ag(b, 0)

                # ================= sparse phases ===========================
                for p in range(L * T):
                    l, t = p // T, p % T
                    width = H if t == 0 else KH
                    tabs = ztab_q if t == 0 else ytab_q
                    rw0 = (l * T + t) * KH

                    def evict(b, hi):
                        jh = b - half_base[hi]
                        asl = acc_sb[:, jh * KH:jh * KH + width]
                        ps = pp_root.tile([128, KH], F32, tag="root")
                        # root (full width, starts the psum group)
                        nc.tensor.matmul(ps[:], hT_sb[:, b * 128:(b + 1) * 128],
                                         rootw_sb[:, rw0:rw0 + KH],
                                         start=True, stop=False)
                        if t == 0:
                            trp = pp_tr.tile([128, 256], BF16, tag="tr")
                            nc.tensor.transpose(trp[:, :128], asl,
                                                ident_b[:])
                            accT = epool.tile([128, 128], BF16, tag="accT")
                            nc.scalar.activation(accT[:],
                                                 trp[:, :128],
                                                 ACT.Copy)
                            nc.tensor.matmul(ps[:], accT[:],
                                             initw_sb[:, l * KH:(l + 1) * KH],
                                             start=False, stop=False)
                        else:
                            trp = pp_tr.tile([128, 256], BF16, tag="tr")
                            for k in range(K):
                                ksl = acc_sb[:, jh * KH + k * H:
                                             jh * KH + (k + 1) * H]
                                tsl = trp[:, k * H:(k + 1) * H]
                                nc.tensor.transpose(tsl, ksl,
                                                    ident_b[:])
                                accT = epool.tile([128, 128], BF16, tag="accT")
                                nc.scalar.activation(accT[:],
                                                     tsl,
                                                     ACT.Copy)
                                wo = (l * max(1, T - 1) * K + k) * H
                                nc.tensor.matmul(ps[:, k * H:(k + 1) * H],
                                                 accT[:], wmat_sb[:, wo:wo + H],
                                                 start=False, stop=False,
                                                 skip_group_check=True)
                        nc.tensor.matmul(ps[:], ones_row[:],
                                         bias_sb[:, rw0:rw0 + KH],
                                         start=False, stop=True,
                                         skip_group_check=True)
                        osb = epool.tile([128, KH], BF16, tag="osb")
                        nc.scalar.activation(osb[:], ps[:], ACT.Relu)
                        if debug and _rep == 0 and p == 0:
                            dba = epool.tile([128, H], F32, tag="dba")
                            nc.vector.tensor_copy(out=dba[:], in_=asl)
                            nc.sync.dma_start(
                                out=dbg_agg[b * 128:(b + 1) * 128, :],
                                in_=dba[:])
                            nc.sync.dma_start(
                                out=dbg_out0[b * 128:(b + 1) * 128, :],
                                in_=osb[:])
                        q, r0 = qb_of(b)
                        if t == 0:
                            # table for t=1 phase: y = out0
                            nc.sync.dma_start(out=yin_q[q][r0:r0 + 128, :],
                                              in_=osb[:])
                            maybe_ag(b, p + 1)
                        else:
                            # h_next = relu(mean_k out)
                            hs = epool.tile([128, H], BF16, tag="hs")
                            nc.vector.tensor_tensor(out=hs[:], in0=osb[:, :H],
                                                    in1=osb[:, H:KH],
                                                    op=ALU.add)
                            hr = epool.tile([128, H], BF16, tag="hr")
                            nc.scalar.activation(hr[:], hs[:], ACT.Relu,
                                                 scale=1.0 / K)
                            if p < L * T - 1:
                                nc.sync.dma_start(out=zin_q[q][r0:r0 + 128, :],
                                                  in_=hr[:])
                                maybe_ag(b, p + 1)
                            trp = pp_tr.tile([128, 128], BF16, tag="tr")
                            nc.tensor.transpose(trp[:], hr[:], ident_b[:])
                            nc.scalar.activation(
                                hT_sb[:, b * 128:(b + 1) * 128],
                                trp[:], ACT.Copy)
                            if p == L * T - 1:
                                lp = pp_root.tile([128, KH], F32, tag="root")
                                nc.tensor.matmul(
                                    lp[:, :C],
                                    hT_sb[:, b * 128:(b + 1) * 128],
                                    wout_sb[:], start=True, stop=False)
                                nc.tensor.matmul(lp[:, :C], ones_row[:],
                                                 bout_sb[:], start=False,
                                                 stop=True,
                                                 skip_group_check=True)
                                ls = epool.tile([128, C], F32, tag="ls")
                                nc.vector.tensor_copy(out=ls[:], in_=lp[:, :C])
                                nc.sync.dma_start(
                                    out=logits[b * 128:(b + 1) * 128, :],
                                    in_=ls[:])

                    cap = 4 if width == H else 2  # blocks packed per PSUM bank
                    for hi, half in enumerate(dv.halves):
                        for pair in range(2):
                            for g in half:
                                blocks = dv.groups[g]
                                psums = {}

                                def agg_slice(j):
                                    pt = psums[j // cap]
                                    return pt[:, (j % cap) * width:
                                              (j % cap + 1) * width]

                                for qi, q in enumerate((2 * pair,
                                                        2 * pair + 1)):
                                    _, slot_base, tile_base = call_info[(g, q)]
                                    nt = len(blocks) * TGC
                                    gt = gpool.tile([128, GB * TGC, width],
                                                    BF16, tag="gath")
                                    if not no_gathers:
                                        nc.gpsimd.dma_gather(
                                            gt[:, :nt, :], tabs[q][:],
                                            gidx_sb[:, slot_base // 16:
                                                    (slot_base + nt * 128) // 16],
                                            num_idxs=nt * 128,
                                            num_idxs_reg=nidx_reg(nt * 128),
                                            elem_size=width,
                                            single_packet=False,
                                            queue_num=gq_counter[0] % 4)
                                        gq_counter[0] += 1
                                    mt = m_bulk(tile_base, nt)
                                    for j, b in enumerate(blocks):
                                        if qi == 0 and j % cap == 0:
                                            psums[j // cap] = pp_agg.tile(
                                                [128, 512], F32, tag="agg",
                                                name=f"agg_{_rep}_{p}_{g}"
                                                     f"_{pair}_{j // cap}")
                                        # start=True clears has_written for
                                        # the WHOLE bank: only the first
                                        # matmul of each bank may set it.
                                        jlast = min(j // cap * cap + cap - 1,
                                                    len(blocks) - 1)
                                        for t2 in range(TGC):
                                            nc.tensor.matmul(
                                                agg_slice(j),
                                                mt[:, j * TGC + t2, :],
                                                gt[:, j * TGC + t2, :],
                                                start=(qi == 0 and t2 == 0
                                                       and j % cap == 0),
                                                stop=(qi == 1 and
                                                      t2 == TGC - 1 and
                                                      j == jlast),
                                                skip_group_check=True)
                                for j, b in enumerate(blocks):
                                    jh = b - half_base[hi]
                                    asl = acc_sb[:, jh * KH:jh * KH + width]
                                    if pair == 0:
                                        nc.vector.tensor_copy(
                                            out=asl, in_=agg_slice(j))
                                    else:
                                        nc.vector.tensor_tensor(
                                            out=asl, in0=asl,
                                            in1=agg_slice(j),
                                            op=ALU.add)
                                        evict(b, hi)

    lower_extended_insts(nc)
    return nc


_CACHE = {}


def _get_built(cfg, TGC, repeat=1, debug=False, **kw):
    key = (cfg, TGC, repeat, debug, tuple(sorted(kw.items())))
    if key not in _CACHE:
        _CACHE[key] = build_nc(cfg, derive(cfg), TGC, repeat=repeat,
                               debug=debug, **kw)
    return _CACHE[key]


def run_on_hw(cfg, inputs, trace=False, debug=False):
    from concourse.bass_utils import run_bass_kernel_spmd

    dv = derive(cfg)
    per_core, TGC = pack_host(cfg, dv, **inputs)
    nc = _get_built(cfg, TGC, repeat=1, debug=debug)
    if not getattr(nc, "_waits_fixed", False):
        fix_excess_waits(nc)
        nc._waits_fixed = True
    res = run_bass_kernel_spmd(nc, per_core, list(range(cfg.W)), trace=trace)
    out = np.concatenate(
        [res.results[c]["logits"][:dv.NPper] for c in range(cfg.W)], axis=0
    )
    return out, res


def kernel(**inputs) -> np.ndarray:
    cfg = Cfg()
    out, _ = run_on_hw(cfg, inputs)
    return out.astype(np.float32)



# revision 35
# speedup vs baseline: 1.2000x; 1.2000x over previous
"""ARMA GNN (nn_ARMA_85976655332070) Trainium2 Bass kernel, 8 NeuronCores.

Strategy v3 (graph/data parallel per sharding hint, optimized):
 - Nodes sharded contiguously across 8 cores (12500/core, padded to 12544).
 - gcn_norm folded into per-edge weights on host (structure-only float
   preprocessing); no deg/dis pass on device.
 - Linearity trick: segment_sum commutes with the per-stack feature
   transforms, so each phase communicates the UNTRANSFORMED node table
   (width H for t=0 phases, K*H for t=1) and applies init_w / w AFTER
   aggregation, per dst block. Halves t=0 traffic and removes the P0 pass.
 - All tables / messages / one-hot tiles in bf16 (halves DMA + AllGather
   bytes); PSUM accumulation stays f32; dense weights bf16.
 - Edges grouped by (dst-block, src-quarter); per sparse phase the work is
   ordered half -> quarter-pair -> dst-group(4 blocks) -> quarter, with
   PSUM accumulation across the quarter pair and bf16 SBUF accumulators
   across pairs, so the in-order Pool engine never stalls behind the last
   AllGather and evictions stream early (launching next-phase AllGathers).

 v3 perf notes (measured on HW, 9.65ms -> ~5.0ms):
 - dma_gather is DESCRIPTOR-RATE bound (~8.25ns/desc on one SWDGE queue,
   independent of elem width or core count). Spreading gather calls
   round-robin over 4 SWDGE queues (num_swdge_queues=4) runs them at
   ~2.2-3ns/desc; gathers-only drops 8.3ms -> 4.5ms. single_packet=True
   hard-wedges the device (NRT_EXEC_UNIT_UNRECOVERABLE) - never use.
 - AllGathers are nearly free (~0.35ms total for all 16); do not optimize.
 - One-hot m tiles are built per gather call in 2 big DVE tensor_tensor
   ops (iota/dl/norm broadcast_to views) instead of per-tile tensor_scalar
   (20x fewer DVE instructions).
 - Aggregation psums are packed 4 (t=0) / 2 (t=1) dst blocks per PSUM
   bank. PSUM rule: start=True clears has_written for the WHOLE bank, so
   only the first matmul touching a bank sets start; later blocks' first
   writes rely on per-element overwrite-where-unset.
 - Cross-pair accumulator acc_sb is bf16 (frees 26KB/partition -> gather
   pool 6 bufs deep, m pool 5) at ~+0.04% rel err (0.0054 -> 0.0057).

 Measured dead ends (do not retry):
 - fp8e4 y-tables: pure fp8 gathers are SLOWER than bf16-512B (5.6ms vs
   4.5ms gathers-only) - 1-byte dtype is bad on the SWDGE gather path.
 - GB=8 (fewer, bigger gather calls): 6.1ms - the forced shallower
   gather/m pools (SBUF) and 4-banks-per-group PSUM pressure at t=1
   outweigh the halved 994ns/call descgen fixed cost.
 - single_packet=True: wedges the device (NRT_EXEC_UNIT_UNRECOVERABLE).
"""

import math
import sys
from dataclasses import dataclass

import numpy as np
import ml_dtypes

sys.path.insert(0, "/opt/trn_rl_repo")

import bass_rust  # noqa: E402
import concourse.bass as bass  # noqa: E402
import concourse.mybir as mybir  # noqa: E402
import concourse.tile as tile  # noqa: E402
from concourse import library_config  # noqa: E402
from concourse.library_overlay import lower_extended_insts  # noqa: E402

F32 = mybir.dt.float32
F32R = mybir.dt.float32r
BF16 = mybir.dt.bfloat16
F8 = mybir.dt.float8e4
I16 = mybir.dt.int16
ALU = mybir.AluOpType
ACT = mybir.ActivationFunctionType
BFNP = ml_dtypes.bfloat16


def fix_excess_waits(nc, limit=1):
    """This walrus build allows very few sync-waits per instruction; move the
    excess onto single-wait EventSemaphore carriers placed just before."""
    n = 0
    for f in nc.m.functions:
        for bb in f.blocks:
            out = []
            for inst in bb.instructions:
                si = inst.sync_info
                if si is not None and len(si.on_wait) > limit:
                    waits = list(si.on_wait)
                    keep = [] if type(inst).__name__ == "InstDrain" else waits[:limit]
                    for w in waits[len(keep):]:
                        ev = bass_rust.InstEventSemaphore(
                            name=f"wsplit_{n}", engine=inst.engine, ins=[], outs=[]
                        )
                        n += 1
                        ev.sync_info = bass_rust.SyncInfo(on_wait=[w], on_update=[])
                        out.append(ev)
                    inst.sync_info = bass_rust.SyncInfo(
                        on_wait=keep, on_update=list(si.on_update)
                    )
                out.append(inst)
            bb.instructions = out
    return n


@dataclass(frozen=True)
class Cfg:
    N: int = 100_000
    F: int = 512
    H: int = 128
    K: int = 2
    T: int = 2
    L: int = 2
    C: int = 40
    W: int = 8  # cores
    GB: int = 4  # dst blocks per matmul group


@dataclass(frozen=True)
class Derived:
    NPper: int
    NB: int
    NPAD: int
    QBLK: int
    qstart: tuple
    qrows: tuple
    groups: tuple
    halves: tuple
    FB: int
    KH: int


def derive(cfg: Cfg) -> Derived:
    assert cfg.N % cfg.W == 0 and cfg.H == 128 and cfg.F % 128 == 0
    NPper = cfg.N // cfg.W
    NB = (NPper + 127) // 128
    NPAD = NB * 128
    QBLK = (NB + 3) // 4
    qstart = tuple(min(i * QBLK, NB) for i in range(5))
    qrows = tuple((qstart[i + 1] - qstart[i]) * 128 for i in range(4))
    assert all(cfg.W * qr <= 32767 for qr in qrows), "int16 gather idx overflow"
    groups = tuple(tuple(range(i, min(i + cfg.GB, NB)))
                   for i in range(0, NB, cfg.GB))
    NG = len(groups)
    halves = (tuple(range(0, (NG + 1) // 2)), tuple(range((NG + 1) // 2, NG)))
    return Derived(NPper, NB, NPAD, QBLK, qstart, qrows, groups, halves,
                   cfg.F // 128, cfg.K * cfg.H)


def call_layout(cfg, dv, TGC):
    """Static per-core call order (half, pair, group, quarter)."""
    GT = TGC * 128
    calls = []
    slot = 0
    for half in dv.halves:
        for pair in range(2):
            for g in half:
                for q in (2 * pair, 2 * pair + 1):
                    blocks = dv.groups[g]
                    calls.append((g, q, blocks, slot, slot // 128))
                    slot += len(blocks) * GT
    return calls, slot


def pack_host(cfg, dv, x, edge_index, edge_weight, Win, b_in, init_w, w,
              root_w, bias, Wout, b_out):
    """Host preprocessing: gcn_norm on edge weights (float32), index packing,
    padding/transposes, bf16 weight conversion."""
    W, NPper, NB, NPAD, GB = cfg.W, dv.NPper, dv.NB, dv.NPAD, cfg.GB
    src = np.asarray(edge_index[0], dtype=np.int64)
    dst = np.asarray(edge_index[1], dtype=np.int64)
    ew = np.asarray(edge_weight, dtype=np.float32)
    E = src.shape[0]

    deg = np.zeros(cfg.N, np.float32)
    np.add.at(deg, dst, ew)
    dis = np.where(deg > 0, 1.0 / np.sqrt(np.maximum(deg, 1e-30)), 0.0)
    dis = dis.astype(np.float32)
    norm = (dis[src] * ew * dis[dst]).astype(np.float32)

    c = dst // NPper
    dloc = dst - c * NPper
    b = dloc >> 7
    dl = (dloc & 127).astype(np.float32)
    rs = src // NPper
    ns = src - rs * NPper
    bs = ns >> 7
    qe = np.minimum(bs // dv.QBLK, 3)
    qs = np.asarray(dv.qstart)[qe] * 128
    row = rs * np.asarray(dv.qrows)[qe] + (ns - qs)

    counts = np.zeros((W, NB, 4), np.int64)
    np.add.at(counts, (c, b, qe), 1)
    TGC = max(1, int(math.ceil(counts.max() / 128.0)))
    GT = TGC * 128

    calls, SLOTS = call_layout(cfg, dv, TGC)
    NTILES = SLOTS // 128
    NG = len(dv.groups)
    cb_arr = np.zeros((NG, 4), np.int64)
    for (g, q, blocks, slot_base, tile_base) in calls:
        cb_arr[g, q] = slot_base

    g_of_b = np.arange(NB) // GB
    j_of_b = np.arange(NB) % GB

    key = (c * NB + b) * 4 + qe
    counts_flat = np.zeros(W * NB * 4, np.int64)
    np.add.at(counts_flat, key, 1)
    starts = np.zeros_like(counts_flat)
    starts[1:] = np.cumsum(counts_flat)[:-1]
    order = np.argsort(key, kind="stable")
    rank = np.empty(E, np.int64)
    rank[order] = np.arange(E) - starts[key[order]]

    slot_lin = cb_arr[g_of_b[b], qe] + j_of_b[b] * GT + rank

    gidx16 = np.zeros((W, 16, SLOTS // 16), np.int16)
    gidx16[c, slot_lin % 16, slot_lin // 16] = row.astype(np.int16)
    gidx = np.tile(gidx16, (1, 8, 1))

    tc_edge = (cb_arr[g_of_b[b], qe] >> 7) + j_of_b[b] * TGC + (rank >> 7)
    p_edge = rank & 127
    gdl = np.zeros((W, 128, NTILES), BFNP)
    gdl[c, p_edge, tc_edge] = dl
    gnw = np.zeros((W, 128, NTILES), np.float32)
    gnw[c, p_edge, tc_edge] = norm
    gnw = gnw.astype(BFNP)
    
    # x: pad + per-core transpose -> [W, FB, 128, NPAD] (f32)
    x = np.asarray(x, dtype=np.float32)
    xpad = np.zeros((W, NPAD, cfg.F), np.float32)
    xpad[:, :NPper, :] = x.reshape(W, NPper, cfg.F)
    xT4 = np.ascontiguousarray(
        xpad.reshape(W, NPAD, dv.FB, 128).transpose(0, 2, 3, 1))

    K, H = cfg.K, cfg.H
    win4 = np.ascontiguousarray(
        np.asarray(Win, np.float32).reshape(dv.FB, 128, H))
    initw2 = np.ascontiguousarray(
        np.asarray(init_w, np.float32).transpose(0, 2, 1, 3)
        .reshape(cfg.L, H, K * H)).astype(BFNP)
    rootw2 = np.ascontiguousarray(
        np.asarray(root_w, np.float32).transpose(0, 1, 3, 2, 4)
        .reshape(cfg.L, cfg.T, H, K * H)).astype(BFNP)
    bias2 = np.ascontiguousarray(
        np.asarray(bias, np.float32).reshape(cfg.L, cfg.T, 1, K * H)
    ).astype(BFNP)
    wmat = np.ascontiguousarray(np.asarray(w, np.float32)).astype(BFNP)
    wout = np.asarray(Wout, np.float32).astype(BFNP)
    bout = np.asarray(b_out, np.float32).reshape(1, cfg.C).astype(BFNP)
    b_in_col = np.asarray(b_in, np.float32).reshape(H, 1)
    iota = np.tile(np.arange(128, dtype=np.float32), (128, 1)).astype(BFNP)

    per_core = []
    for ci in range(W):
        per_core.append(dict(
            xT4=xT4[ci], gidx=gidx[ci], gdl=gdl[ci], gnw=gnw[ci],
            win4=win4, b_in_col=b_in_col, initw2=initw2, rootw2=rootw2,
            bias2=bias2, wmat=wmat, wout=wout, bout=bout, iota=iota,
        ))
    return per_core, TGC


def build_nc(cfg: Cfg, dv: Derived, TGC: int, repeat: int = 1,
             debug: bool = False, no_collectives: bool = False,
             no_gathers: bool = False):
    from concourse.masks import make_identity

    K, H, T, L, C = cfg.K, cfg.H, cfg.T, cfg.L, cfg.C
    KH, NB, NPAD, FB, GB = dv.KH, dv.NB, dv.NPAD, dv.FB, cfg.GB
    GT = TGC * 128
    calls, SLOTS = call_layout(cfg, dv, TGC)
    NTILES = SLOTS // 128
    MAXHALF = max(sum(len(dv.groups[g]) for g in half) for half in dv.halves)
    half_base = [min(min(dv.groups[g]) for g in half) for half in dv.halves]

    nc = bass.Bass(num_swdge_queues=4)
    # ---- params
    xT4 = nc.declare_dram_parameter("xT4", [FB, 128, NPAD], F32R, isOutput=False)
    gidx = nc.declare_dram_parameter("gidx", [128, SLOTS // 16], I16,
                                     isOutput=False)
    gdl = nc.declare_dram_parameter("gdl", [128, NTILES], BF16,
                                    isOutput=False)
    gnw = nc.declare_dram_parameter("gnw", [128, NTILES], BF16,
                                    isOutput=False)
    win4 = nc.declare_dram_parameter("win4", [FB, 128, H], F32R, isOutput=False)
    b_in_col = nc.declare_dram_parameter("b_in_col", [H, 1], F32,
                                         isOutput=False)
    initw2 = nc.declare_dram_parameter("initw2", [L, H, KH], BF16,
                                       isOutput=False)
    rootw2 = nc.declare_dram_parameter("rootw2", [L, T, H, KH], BF16,
                                       isOutput=False)
    bias2 = nc.declare_dram_parameter("bias2", [L, T, 1, KH], BF16,
                                      isOutput=False)
    wmat = nc.declare_dram_parameter("wmat", [L, max(1, T - 1), K, H, H], BF16,
                                     isOutput=False)
    wout = nc.declare_dram_parameter("wout", [H, C], BF16, isOutput=False)
    bout = nc.declare_dram_parameter("bout", [1, C], BF16, isOutput=False)
    iota_in = nc.declare_dram_parameter("iota", [128, 128], BF16,
                                        isOutput=False)
    logits = nc.declare_dram_parameter("logits", [NPAD, C], F32, isOutput=True)
    if debug:
        dbg_h = nc.declare_dram_parameter("dbg_h", [NPAD, H], BF16,
                                          isOutput=True)
        dbg_agg = nc.declare_dram_parameter("dbg_agg", [NPAD, H], F32,
                                            isOutput=True)
        dbg_out0 = nc.declare_dram_parameter("dbg_out0", [NPAD, KH], BF16,
                                             isOutput=True)

    # ---- internal DRAM: per-quarter z (width H) and y (width KH) tables
    zin_q, ztab_q, yin_q, ytab_q = [], [], [], []
    for q in range(4):
        r = dv.qrows[q]
        zin_q.append(nc.dram_tensor(f"zin_{q}", [r, H], BF16))
        ztab_q.append(nc.dram_tensor(f"ztab_{q}", [cfg.W * r, H], BF16,
                                     addr_space="Shared"))
        yin_q.append(nc.dram_tensor(f"yin_{q}", [r, KH], BF16))
        ytab_q.append(nc.dram_tensor(f"ytab_{q}", [cfg.W * r, KH], BF16,
                                     addr_space="Shared"))

    rg = [list(range(cfg.W))]

    def qb_of(b):
        q = min(b // dv.QBLK, 3)
        return q, (b - dv.qstart[q]) * 128

    with tile.TileContext(nc) as tc:
        nc.gpsimd.load_library(library_config.mlp)
        import contextlib
        ctx = contextlib.ExitStack()
        with ctx:
            cpool = ctx.enter_context(tc.tile_pool(name="const", bufs=1))
            xpool = ctx.enter_context(tc.tile_pool(name="xin", bufs=3))
            gpool = ctx.enter_context(tc.tile_pool(name="gath", bufs=6))
            mpool = ctx.enter_context(tc.tile_pool(name="mbuild", bufs=5))
            ypool = ctx.enter_context(tc.tile_pool(name="ywrite", bufs=3))
            epool = ctx.enter_context(tc.tile_pool(name="evict", bufs=8))
            pp_agg = ctx.enter_context(
                tc.tile_pool(name="pagg", bufs=5, space="PSUM"))
            pp_root = ctx.enter_context(
                tc.tile_pool(name="proot", bufs=2, space="PSUM"))
            pp_tr = ctx.enter_context(
                tc.tile_pool(name="ptr", bufs=1, space="PSUM"))

            # ---- persistent SBUF
            iota_sb = cpool.tile([128, 128], BF16, tag="iota")
            nc.sync.dma_start(out=iota_sb[:], in_=iota_in[:])
            ident_raw = cpool.tile([128, 128], F32, tag="ident_raw")
            make_identity(nc, ident_raw[:])
            ident_f = cpool.tile([128, 128], F32R, tag="ident_f")
            nc.vector.tensor_copy(out=ident_f[:], in_=ident_raw[:])
            ident_b = cpool.tile([128, 128], BF16, tag="ident_b")
            nc.vector.tensor_copy(out=ident_b[:], in_=ident_raw[:])
            ones_row = cpool.tile([1, 128], BF16, tag="ones_row")
            nc.gpsimd.memset(ones_row[:], 1.0)
            binc = cpool.tile([H, 1], F32, tag="binc")
            nc.sync.dma_start(out=binc[:], in_=b_in_col[:])
            win_sb = cpool.tile([128, FB * H], F32R, tag="win")
            for fb in range(FB):
                nc.sync.dma_start(out=win_sb[:, fb * H:(fb + 1) * H],
                                  in_=win4[fb])
            initw_sb = cpool.tile([128, L * KH], BF16, tag="initw")
            for l in range(L):
                nc.sync.dma_start(out=initw_sb[:, l * KH:(l + 1) * KH],
                                  in_=initw2[l])
            rootw_sb = cpool.tile([128, L * T * KH], BF16, tag="rootw")
            for l in range(L):
                for t in range(T):
                    o = (l * T + t) * KH
                    nc.sync.dma_start(out=rootw_sb[:, o:o + KH],
                                      in_=rootw2[l, t])
            bias_sb = cpool.tile([1, L * T * KH], BF16, tag="bias")
            for l in range(L):
                for t in range(T):
                    o = (l * T + t) * KH
                    nc.sync.dma_start(out=bias_sb[:, o:o + KH], in_=bias2[l, t])
            wmat_sb = cpool.tile([128, L * max(1, T - 1) * K * H], BF16,
                                 tag="wmat")
            for l in range(L):
                for t in range(max(1, T - 1)):
                    for k in range(K):
                        o = ((l * max(1, T - 1) + t) * K + k) * H
                        nc.sync.dma_start(out=wmat_sb[:, o:o + H],
                                          in_=wmat[l, t, k])
            wout_sb = cpool.tile([H, C], BF16, tag="wout")
            nc.sync.dma_start(out=wout_sb[:], in_=wout[:])
            bout_sb = cpool.tile([1, C], BF16, tag="bout")
            nc.sync.dma_start(out=bout_sb[:], in_=bout[:])
            gidx_sb = cpool.tile([128, SLOTS // 16], I16, tag="gidx")
            nc.sync.dma_start(out=gidx_sb[:], in_=gidx[:])
            gdl_sb = cpool.tile([128, NTILES], BF16, tag="gdl")
            nc.sync.dma_start(out=gdl_sb[:], in_=gdl[:])
            gnw_sb = cpool.tile([128, NTILES], BF16, tag="gnw")
            nc.sync.dma_start(out=gnw_sb[:], in_=gnw[:])
            hT_sb = cpool.tile([128, NPAD], BF16, tag="hT")
            acc_sb = cpool.tile([128, MAXHALF * KH], BF16, tag="acc")

            _nidx_regs = {}

            def nidx_reg(v):
                if v not in _nidx_regs:
                    _nidx_regs[v] = nc.gpsimd.to_reg(v)
                return _nidx_regs[v]

            def m_bulk(tile_base, nt):
                """One-hot-times-norm tiles for a whole call in 2 DVE ops."""
                mt = mpool.tile([128, cfg.GB * TGC, 128], BF16, tag="m")
                msl = mt[:, :nt, :]
                iota_b = iota_sb[:].unsqueeze(1).broadcast_to([128, nt, 128])
                dl_b = gdl_sb[:, tile_base:tile_base + nt].unsqueeze(2) \
                    .broadcast_to([128, nt, 128])
                nw_b = gnw_sb[:, tile_base:tile_base + nt].unsqueeze(2) \
                    .broadcast_to([128, nt, 128])
                nc.vector.tensor_tensor(out=msl, in0=iota_b, in1=dl_b,
                                        op=ALU.is_equal)
                nc.vector.tensor_tensor(out=msl, in0=msl, in1=nw_b,
                                        op=ALU.mult)
                return mt

            call_info = {(g, q): (blocks, slot_base, tile_base)
                         for (g, q, blocks, slot_base, tile_base) in calls}
            gq_counter = [0]

            for _rep in range(repeat):
                def maybe_ag(b, p):
                    # launch quarter AllGather for phase p once its last
                    # block's table rows have been written
                    if p >= L * T:
                        return
                    for q in range(4):
                        if b == dv.qstart[q + 1] - 1:
                            if p % 2 == 0:
                                ins, outs = zin_q[q], ztab_q[q]
                            else:
                                ins, outs = yin_q[q], ytab_q[q]
                            if not no_collectives:
                                nc.gpsimd.collective_compute(
                                    "AllGather", ALU.bypass, replica_groups=rg,
                                    ins=[ins[:]], outs=[outs[:]])

                # ========== input: hT = relu(Win^T xT + b_in); z0 = h ======
                for g0 in range(0, NPAD, 256):
                    gs = min(256, NPAD - g0)
                    ps = pp_root.tile([128, KH], F32, tag="root")
                    for fb in range(FB):
                        xt = xpool.tile([128, 256], F32R, tag="xin")
                        nc.sync.dma_start(out=xt[:, :gs],
                                          in_=xT4[fb, :, g0:g0 + gs])
                        nc.tensor.matmul(ps[:, :gs],
                                         win_sb[:, fb * H:(fb + 1) * H],
                                         xt[:, :gs],
                                         start=(fb == 0), stop=(fb == FB - 1))
                    nc.scalar.activation(hT_sb[:, g0:g0 + gs], ps[:, :gs],
                                         ACT.Relu, bias=binc[:])
                    for b in range(g0 // 128, (g0 + gs) // 128):
                        bsl = slice(b * 128, (b + 1) * 128)
                        trp = pp_tr.tile([128, 128], BF16, tag="tr")
                        nc.tensor.transpose(trp[:], hT_sb[:, bsl], ident_b[:])
                        zt = ypool.tile([128, KH], BF16, tag="y")
                        nc.vector.tensor_copy(out=zt[:, :H], in_=trp[:])
                        q, r0 = qb_of(b)
                        nc.sync.dma_start(out=zin_q[q][r0:r0 + 128, :],
                                          in_=zt[:, :H])
                        if debug and _rep == 0:
                            nc.sync.dma_start(
                                out=dbg_h[b * 128:(b + 1) * 128, :],
                                in_=zt[:, :H])
                        maybe_ag(b, 0)

                # ================= sparse phases ===========================
                for p in range(L * T):
                    l, t = p // T, p % T
                    width = H if t == 0 else KH
                    tabs = ztab_q if t == 0 else ytab_q
                    rw0 = (l * T + t) * KH

                    def evict(b, hi):
                        jh = b - half_base[hi]
                        asl = acc_sb[:, jh * KH:jh * KH + width]
                        ps = pp_root.tile([128, KH], F32, tag="root")
                        # root (full width, starts the psum group)
                        nc.tensor.matmul(ps[:], hT_sb[:, b * 128:(b + 1) * 128],
                                         rootw_sb[:, rw0:rw0 + KH],
                                         start=True, stop=False)
                        if t == 0:
                            trp = pp_tr.tile([128, 256], BF16, tag="tr")
                            nc.tensor.transpose(trp[:, :128], asl,
                                                ident_b[:])
                            accT = epool.tile([128, 128], BF16, tag="accT")
                            nc.scalar.activation(accT[:],
                                                 trp[:, :128],
                                                 ACT.Copy)
                            nc.tensor.matmul(ps[:], accT[:],
                                             initw_sb[:, l * KH:(l + 1) * KH],
                                             start=False, stop=False)
                        else:
                            trp = pp_tr.tile([128, 256], BF16, tag="tr")
                            for k in range(K):
                                ksl = acc_sb[:, jh * KH + k * H:
                                             jh * KH + (k + 1) * H]
                                tsl = trp[:, k * H:(k + 1) * H]
                                nc.tensor.transpose(tsl, ksl,
                                                    ident_b[:])
                                accT = epool.tile([128, 128], BF16, tag="accT")
                                nc.scalar.activation(accT[:],
                                                     tsl,
                                                     ACT.Copy)
                                wo = (l * max(1, T - 1) * K + k) * H
                                nc.tensor.matmul(ps[:, k * H:(k + 1) * H],
                                                 accT[:], wmat_sb[:, wo:wo + H],
                                                 start=False, stop=False,
                                                 skip_group_check=True)
                        nc.tensor.matmul(ps[:], ones_row[:],
                                         bias_sb[:, rw0:rw0 + KH],
                                         start=False, stop=True,
                                         skip_group_check=True)
                        osb = epool.tile([128, KH], BF16, tag="osb")
                        nc.scalar.activation(osb[:], ps[:], ACT.Relu)
                        if debug and _rep == 0 and p == 0:
                            dba = epool.tile([128, H], F32, tag="dba")
                            nc.vector.tensor_copy(out=dba[:], in_=asl)
                            nc.sync.dma_start(
                                out=dbg_agg[b * 128:(b + 1) * 128, :],
                                in_=dba[:])
                            nc.sync.dma_start(
                                out=dbg_out0[b * 128:(b + 1) * 128, :],
                                in_=osb[:])
                        q, r0 = qb_of(b)
                        if t == 0:
                            # table for t=1 phase: y = out0
                            nc.sync.dma_start(out=yin_q[q][r0:r0 + 128, :],
                                              in_=osb[:])
                            maybe_ag(b, p + 1)
                        else:
                            # h_next = relu(mean_k out)
                            hs = epool.tile([128, H], BF16, tag="hs")
                            nc.vector.tensor_tensor(out=hs[:], in0=osb[:, :H],
                                                    in1=osb[:, H:KH],
                                                    op=ALU.add)
                            hr = epool.tile([128, H], BF16, tag="hr")
                            nc.scalar.activation(hr[:], hs[:], ACT.Relu,
                                                 scale=1.0 / K)
                            if p < L * T - 1:
                                nc.sync.dma_start(out=zin_q[q][r0:r0 + 128, :],
                                                  in_=hr[:])
                                maybe_ag(b, p + 1)
                            trp = pp_tr.tile([128, 128], BF16, tag="tr")
                            nc.tensor.transpose(trp[:], hr[:], ident_b[:])
                            nc.scalar.activation(
                                hT_sb[:, b * 128:(b + 1) * 128],
                                trp[:], ACT.Copy)
                            if p == L * T - 1:
                                lp = pp_root.tile([128, KH], F32, tag="root")
                                nc.tensor.matmul(
                                    lp[:, :C],
                                    hT_sb[:, b * 128:(b + 1) * 128],
                                    wout_sb[:], start=True, stop=False)
                                nc.tensor.matmul(lp[:, :C], ones_row[:],
                                                 bout_sb[:], start=False,
                                                 stop=True,
                                                 skip_group_check=True)
                                ls = epool.tile([128, C], F32, tag="ls")
                                nc.vector.tensor_copy(out=ls[:], in_=lp[:, :C])
                                nc.sync.dma_start(
                                    out=logits[b * 128:(b + 1) * 128, :],
                                    in_=ls[:])

                    cap = 4 if width == H else 2  # blocks packed per PSUM bank
                    for hi, half in enumerate(dv.halves):
                        for pair in range(2):
                            for g in half:
                                blocks = dv.groups[g]
                                psums = {}

                                def agg_slice(j):
                                    pt = psums[j // cap]
                                    return pt[:, (j % cap) * width:
                                              (j % cap + 1) * width]

                                for qi, q in enumerate((2 * pair,
                                                        2 * pair + 1)):
                                    _, slot_base, tile_base = call_info[(g, q)]
                                    nt = len(blocks) * TGC
                                    gt = gpool.tile([128, GB * TGC, width],
                                                    BF16, tag="gath")
                                    if not no_gathers:
                                        nc.gpsimd.dma_gather(
                                            gt[:, :nt, :], tabs[q][:],
                                            gidx_sb[:, slot_base // 16:
                                                    (slot_base + nt * 128) // 16],
                                            num_idxs=nt * 128,
                                            num_idxs_reg=nidx_reg(nt * 128),
                                            elem_size=width,
                                            single_packet=False,
                                            queue_num=gq_counter[0] % 4)
                                        gq_counter[0] += 1
                                    mt = m_bulk(tile_base, nt)
                                    for j, b in enumerate(blocks):
                                        if qi == 0 and j % cap == 0:
                                            psums[j // cap] = pp_agg.tile(
                                                [128, 512], F32, tag="agg",
                                                name=f"agg_{_rep}_{p}_{g}"
                                                     f"_{pair}_{j // cap}")
                                        # start=True clears has_written for
                                        # the WHOLE bank: only the first
                                        # matmul of each bank may set it.
                                        jlast = min(j // cap * cap + cap - 1,
                                                    len(blocks) - 1)
                                        for t2 in range(TGC):
                                            nc.tensor.matmul(
                                                agg_slice(j),
                                                mt[:, j * TGC + t2, :],
                                                gt[:, j * TGC + t2, :],
                                                start=(qi == 0 and t2 == 0
                                                       and j % cap == 0),
                                                stop=(qi == 1 and
                                                      t2 == TGC - 1 and
                                                      j == jlast),
                                                skip_group_check=True)
                                for j, b in enumerate(blocks):
                                    jh = b - half_base[hi]
                                    asl = acc_sb[:, jh * KH:jh * KH + width]
                                    if pair == 0:
                                        nc.vector.tensor_copy(
                                            out=asl, in_=agg_slice(j))
                                    else:
                                        nc.vector.tensor_tensor(
                                            out=asl, in0=asl,
                                            in1=agg_slice(j),
                                            op=ALU.add)
                                        evict(b, hi)

    lower_extended_insts(nc)
    return nc


_CACHE = {}


def _get_built(cfg, TGC, repeat=1, debug=False, **kw):
    key = (cfg, TGC, repeat, debug, tuple(sorted(kw.items())))
    if key not in _CACHE:
        _CACHE[key] = build_nc(cfg, derive(cfg), TGC, repeat=repeat,
                               debug=debug, **kw)
    return _CACHE[key]


def run_on_hw(cfg, inputs, trace=False, debug=False):
    from concourse.bass_utils import run_bass_kernel_spmd

    dv = derive(cfg)
    per_core, TGC = pack_host(cfg, dv, **inputs)
    nc = _get_built(cfg, TGC, repeat=1, debug=debug)
    if not getattr(nc, "_waits_fixed", False):
        fix_excess_waits(nc)
        nc._waits_fixed = True
    res = run_bass_kernel_spmd(nc, per_core, list(range(cfg.W)), trace=trace)
    out = np.concatenate(
        [res.results[c]["logits"][:dv.NPper] for c in range(cfg.W)], axis=0
    )
    return out, res


def kernel(**inputs) -> np.ndarray:
    cfg = Cfg()
    out, _ = run_on_hw(cfg, inputs)
    return out.astype(np.float32)



# revision 36
# speedup vs baseline: 1.2486x; 1.0405x over previous
"""ARMA GNN (nn_ARMA_85976655332070) Trainium2 Bass kernel, 8 NeuronCores.

Strategy v3 (graph/data parallel per sharding hint, optimized):
 - Nodes sharded contiguously across 8 cores (12500/core, padded to 12544).
 - gcn_norm folded into per-edge weights on host (structure-only float
   preprocessing); no deg/dis pass on device.
 - Linearity trick: segment_sum commutes with the per-stack feature
   transforms, so each phase communicates the UNTRANSFORMED node table
   (width H for t=0 phases, K*H for t=1) and applies init_w / w AFTER
   aggregation, per dst block. Halves t=0 traffic and removes the P0 pass.
 - All tables / messages / one-hot tiles in bf16 (halves DMA + AllGather
   bytes); PSUM accumulation stays f32; dense weights bf16.
 - Edges grouped by (dst-block, src-quarter); per sparse phase the work is
   ordered half -> quarter-pair -> dst-group(4 blocks) -> quarter, with
   PSUM accumulation across the quarter pair and bf16 SBUF accumulators
   across pairs, so the in-order Pool engine never stalls behind the last
   AllGather and evictions stream early (launching next-phase AllGathers).

 v3 perf notes (measured on HW, 9.65ms -> ~5.0ms):
 - dma_gather is DESCRIPTOR-RATE bound (~8.25ns/desc on one SWDGE queue,
   independent of elem width or core count). Spreading gather calls
   round-robin over 4 SWDGE queues (num_swdge_queues=4) runs them at
   ~2.2-3ns/desc; gathers-only drops 8.3ms -> 4.5ms. single_packet=True
   hard-wedges the device (NRT_EXEC_UNIT_UNRECOVERABLE) - never use.
 - AllGathers are nearly free (~0.35ms total for all 16); do not optimize.
 - One-hot m tiles are built per gather call in 2 big DVE tensor_tensor
   ops (iota/dl/norm broadcast_to views) instead of per-tile tensor_scalar
   (20x fewer DVE instructions).
 - Aggregation psums are packed 4 (t=0) / 2 (t=1) dst blocks per PSUM
   bank. PSUM rule: start=True clears has_written for the WHOLE bank, so
   only the first matmul touching a bank sets start; later blocks' first
   writes rely on per-element overwrite-where-unset.
 - Cross-pair accumulator acc_sb is bf16 (frees 26KB/partition -> gather
   pool 6 bufs deep, m pool 5) at ~+0.04% rel err (0.0054 -> 0.0057).

 Measured dead ends (do not retry):
 - fp8e4 y-tables: pure fp8 gathers are SLOWER than bf16-512B (5.6ms vs
   4.5ms gathers-only) - 1-byte dtype is bad on the SWDGE gather path.
 - GB=8 (fewer, bigger gather calls): 6.1ms - the forced shallower
   gather/m pools (SBUF) and 4-banks-per-group PSUM pressure at t=1
   outweigh the halved 994ns/call descgen fixed cost.
 - single_packet=True: wedges the device (NRT_EXEC_UNIT_UNRECOVERABLE).
 - All-negative (or mostly-negative) gather index streams: reproducibly
   crash the gather ucode / hang the worker. Rules out per-call dynamic
   true-count gathers via negative-padded tails.
"""

import math
import sys
from dataclasses import dataclass

import numpy as np
import ml_dtypes

sys.path.insert(0, "/opt/trn_rl_repo")

import bass_rust  # noqa: E402
import concourse.bass as bass  # noqa: E402
import concourse.mybir as mybir  # noqa: E402
import concourse.tile as tile  # noqa: E402
from concourse import library_config  # noqa: E402
from concourse.library_overlay import lower_extended_insts  # noqa: E402

F32 = mybir.dt.float32
F32R = mybir.dt.float32r
BF16 = mybir.dt.bfloat16
F8 = mybir.dt.float8e4
I16 = mybir.dt.int16
ALU = mybir.AluOpType
ACT = mybir.ActivationFunctionType
BFNP = ml_dtypes.bfloat16


def fix_excess_waits(nc, limit=1):
    """This walrus build allows very few sync-waits per instruction; move the
    excess onto single-wait EventSemaphore carriers placed just before."""
    n = 0
    for f in nc.m.functions:
        for bb in f.blocks:
            out = []
            for inst in bb.instructions:
                si = inst.sync_info
                if si is not None and len(si.on_wait) > limit:
                    waits = list(si.on_wait)
                    keep = [] if type(inst).__name__ == "InstDrain" else waits[:limit]
                    for w in waits[len(keep):]:
                        ev = bass_rust.InstEventSemaphore(
                            name=f"wsplit_{n}", engine=inst.engine, ins=[], outs=[]
                        )
                        n += 1
                        ev.sync_info = bass_rust.SyncInfo(on_wait=[w], on_update=[])
                        out.append(ev)
                    inst.sync_info = bass_rust.SyncInfo(
                        on_wait=keep, on_update=list(si.on_update)
                    )
                out.append(inst)
            bb.instructions = out
    return n


@dataclass(frozen=True)
class Cfg:
    N: int = 100_000
    F: int = 512
    H: int = 128
    K: int = 2
    T: int = 2
    L: int = 2
    C: int = 40
    W: int = 8  # cores
    GB: int = 4  # dst blocks per matmul group


@dataclass(frozen=True)
class Derived:
    NPper: int
    NB: int
    NPAD: int
    QBLK: int
    qstart: tuple
    qrows: tuple
    groups: tuple
    halves: tuple
    FB: int
    KH: int


def derive(cfg: Cfg) -> Derived:
    assert cfg.N % cfg.W == 0 and cfg.H == 128 and cfg.F % 128 == 0
    NPper = cfg.N // cfg.W
    NB = (NPper + 127) // 128
    NPAD = NB * 128
    QBLK = (NB + 3) // 4
    qstart = tuple(min(i * QBLK, NB) for i in range(5))
    qrows = tuple((qstart[i + 1] - qstart[i]) * 128 for i in range(4))
    assert all(cfg.W * qr <= 32767 for qr in qrows), "int16 gather idx overflow"
    groups = tuple(tuple(range(i, min(i + cfg.GB, NB)))
                   for i in range(0, NB, cfg.GB))
    NG = len(groups)
    halves = (tuple(range(0, (NG + 1) // 2)), tuple(range((NG + 1) // 2, NG)))
    return Derived(NPper, NB, NPAD, QBLK, qstart, qrows, groups, halves,
                   cfg.F // 128, cfg.K * cfg.H)


def call_layout(cfg, dv, TGC):
    """Static per-core call order (half, pair, group, quarter)."""
    GT = TGC * 128
    calls = []
    slot = 0
    for half in dv.halves:
        for pair in range(2):
            for g in half:
                for q in (2 * pair, 2 * pair + 1):
                    blocks = dv.groups[g]
                    calls.append((g, q, blocks, slot, slot // 128))
                    slot += len(blocks) * GT
    return calls, slot


def pack_host(cfg, dv, x, edge_index, edge_weight, Win, b_in, init_w, w,
              root_w, bias, Wout, b_out):
    """Host preprocessing: gcn_norm on edge weights (float32), index packing,
    padding/transposes, bf16 weight conversion."""
    W, NPper, NB, NPAD, GB = cfg.W, dv.NPper, dv.NB, dv.NPAD, cfg.GB
    src = np.asarray(edge_index[0], dtype=np.int64)
    dst = np.asarray(edge_index[1], dtype=np.int64)
    ew = np.asarray(edge_weight, dtype=np.float32)
    E = src.shape[0]

    deg = np.zeros(cfg.N, np.float32)
    np.add.at(deg, dst, ew)
    dis = np.where(deg > 0, 1.0 / np.sqrt(np.maximum(deg, 1e-30)), 0.0)
    dis = dis.astype(np.float32)
    norm = (dis[src] * ew * dis[dst]).astype(np.float32)

    c = dst // NPper
    dloc = dst - c * NPper
    b = dloc >> 7
    dl = (dloc & 127).astype(np.float32)
    rs = src // NPper
    ns = src - rs * NPper
    bs = ns >> 7
    qe = np.minimum(bs // dv.QBLK, 3)
    qs = np.asarray(dv.qstart)[qe] * 128
    row = rs * np.asarray(dv.qrows)[qe] + (ns - qs)

    counts = np.zeros((W, NB, 4), np.int64)
    np.add.at(counts, (c, b, qe), 1)
    TGC = max(1, int(math.ceil(counts.max() / 128.0)))
    GT = TGC * 128

    calls, SLOTS = call_layout(cfg, dv, TGC)
    NTILES = SLOTS // 128
    NG = len(dv.groups)
    cb_arr = np.zeros((NG, 4), np.int64)
    for (g, q, blocks, slot_base, tile_base) in calls:
        cb_arr[g, q] = slot_base

    g_of_b = np.arange(NB) // GB
    j_of_b = np.arange(NB) % GB

    key = (c * NB + b) * 4 + qe
    counts_flat = np.zeros(W * NB * 4, np.int64)
    np.add.at(counts_flat, key, 1)
    starts = np.zeros_like(counts_flat)
    starts[1:] = np.cumsum(counts_flat)[:-1]
    order = np.argsort(key, kind="stable")
    rank = np.empty(E, np.int64)
    rank[order] = np.arange(E) - starts[key[order]]

    slot_lin = cb_arr[g_of_b[b], qe] + j_of_b[b] * GT + rank

    gidx16 = np.zeros((W, 16, SLOTS // 16), np.int16)
    gidx16[c, slot_lin % 16, slot_lin // 16] = row.astype(np.int16)
    gidx = np.tile(gidx16, (1, 8, 1))

    tc_edge = (cb_arr[g_of_b[b], qe] >> 7) + j_of_b[b] * TGC + (rank >> 7)
    p_edge = rank & 127
    gdl = np.zeros((W, 128, NTILES), BFNP)
    gdl[c, p_edge, tc_edge] = dl
    gnw = np.zeros((W, 128, NTILES), np.float32)
    gnw[c, p_edge, tc_edge] = norm
    gnw = gnw.astype(BFNP)
    
    # x: pad + per-core transpose -> [W, FB, 128, NPAD] (f32)
    x = np.asarray(x, dtype=np.float32)
    xpad = np.zeros((W, NPAD, cfg.F), np.float32)
    xpad[:, :NPper, :] = x.reshape(W, NPper, cfg.F)
    xT4 = np.ascontiguousarray(
        xpad.reshape(W, NPAD, dv.FB, 128).transpose(0, 2, 3, 1))

    K, H = cfg.K, cfg.H
    win4 = np.ascontiguousarray(
        np.asarray(Win, np.float32).reshape(dv.FB, 128, H))
    initw2 = np.ascontiguousarray(
        np.asarray(init_w, np.float32).transpose(0, 2, 1, 3)
        .reshape(cfg.L, H, K * H)).astype(BFNP)
    rootw2 = np.ascontiguousarray(
        np.asarray(root_w, np.float32).transpose(0, 1, 3, 2, 4)
        .reshape(cfg.L, cfg.T, H, K * H)).astype(BFNP)
    bias2 = np.ascontiguousarray(
        np.asarray(bias, np.float32).reshape(cfg.L, cfg.T, 1, K * H)
    ).astype(BFNP)
    wmat = np.ascontiguousarray(np.asarray(w, np.float32)).astype(BFNP)
    wout = np.asarray(Wout, np.float32).astype(BFNP)
    bout = np.asarray(b_out, np.float32).reshape(1, cfg.C).astype(BFNP)
    b_in_col = np.asarray(b_in, np.float32).reshape(H, 1)
    iota = np.tile(np.arange(128, dtype=np.float32), (128, 1)).astype(BFNP)

    per_core = []
    for ci in range(W):
        per_core.append(dict(
            xT4=xT4[ci], gidx=gidx[ci], gdl=gdl[ci], gnw=gnw[ci],
            win4=win4, b_in_col=b_in_col, initw2=initw2, rootw2=rootw2,
            bias2=bias2, wmat=wmat, wout=wout, bout=bout, iota=iota,
        ))
    return per_core, TGC


def build_nc(cfg: Cfg, dv: Derived, TGC: int, repeat: int = 1,
             debug: bool = False, no_collectives: bool = False,
             no_gathers: bool = False):
    from concourse.masks import make_identity

    K, H, T, L, C = cfg.K, cfg.H, cfg.T, cfg.L, cfg.C
    KH, NB, NPAD, FB, GB = dv.KH, dv.NB, dv.NPAD, dv.FB, cfg.GB
    GT = TGC * 128
    calls, SLOTS = call_layout(cfg, dv, TGC)
    NTILES = SLOTS // 128
    MAXHALF = max(sum(len(dv.groups[g]) for g in half) for half in dv.halves)
    half_base = [min(min(dv.groups[g]) for g in half) for half in dv.halves]

    nc = bass.Bass(num_swdge_queues=4)
    # ---- params
    xT4 = nc.declare_dram_parameter("xT4", [FB, 128, NPAD], F32R, isOutput=False)
    gidx = nc.declare_dram_parameter("gidx", [128, SLOTS // 16], I16,
                                     isOutput=False)
    gdl = nc.declare_dram_parameter("gdl", [128, NTILES], BF16,
                                    isOutput=False)
    gnw = nc.declare_dram_parameter("gnw", [128, NTILES], BF16,
                                    isOutput=False)
    win4 = nc.declare_dram_parameter("win4", [FB, 128, H], F32R, isOutput=False)
    b_in_col = nc.declare_dram_parameter("b_in_col", [H, 1], F32,
                                         isOutput=False)
    initw2 = nc.declare_dram_parameter("initw2", [L, H, KH], BF16,
                                       isOutput=False)
    rootw2 = nc.declare_dram_parameter("rootw2", [L, T, H, KH], BF16,
                                       isOutput=False)
    bias2 = nc.declare_dram_parameter("bias2", [L, T, 1, KH], BF16,
                                      isOutput=False)
    wmat = nc.declare_dram_parameter("wmat", [L, max(1, T - 1), K, H, H], BF16,
                                     isOutput=False)
    wout = nc.declare_dram_parameter("wout", [H, C], BF16, isOutput=False)
    bout = nc.declare_dram_parameter("bout", [1, C], BF16, isOutput=False)
    iota_in = nc.declare_dram_parameter("iota", [128, 128], BF16,
                                        isOutput=False)
    logits = nc.declare_dram_parameter("logits", [NPAD, C], F32, isOutput=True)
    if debug:
        dbg_h = nc.declare_dram_parameter("dbg_h", [NPAD, H], BF16,
                                          isOutput=True)
        dbg_agg = nc.declare_dram_parameter("dbg_agg", [NPAD, H], F32,
                                            isOutput=True)
        dbg_out0 = nc.declare_dram_parameter("dbg_out0", [NPAD, KH], BF16,
                                             isOutput=True)

    # ---- internal DRAM: per-quarter z (width H) and y (width KH) tables
    zin_q, ztab_q, yin_q, ytab_q = [], [], [], []
    for q in range(4):
        r = dv.qrows[q]
        zin_q.append(nc.dram_tensor(f"zin_{q}", [r, H], BF16))
        ztab_q.append(nc.dram_tensor(f"ztab_{q}", [cfg.W * r, H], BF16,
                                     addr_space="Shared"))
        yin_q.append(nc.dram_tensor(f"yin_{q}", [r, KH], BF16))
        ytab_q.append(nc.dram_tensor(f"ytab_{q}", [cfg.W * r, KH], BF16,
                                     addr_space="Shared"))

    rg = [list(range(cfg.W))]

    def qb_of(b):
        q = min(b // dv.QBLK, 3)
        return q, (b - dv.qstart[q]) * 128

    with tile.TileContext(nc) as tc:
        nc.gpsimd.load_library(library_config.mlp)
        import contextlib
        ctx = contextlib.ExitStack()
        with ctx:
            cpool = ctx.enter_context(tc.tile_pool(name="const", bufs=1))
            xpool = ctx.enter_context(tc.tile_pool(name="xin", bufs=3))
            gpool = ctx.enter_context(tc.tile_pool(name="gath", bufs=6))
            mpool = ctx.enter_context(tc.tile_pool(name="mbuild", bufs=5))
            ypool = ctx.enter_context(tc.tile_pool(name="ywrite", bufs=3))
            epool = ctx.enter_context(tc.tile_pool(name="evict", bufs=8))
            pp_agg = ctx.enter_context(
                tc.tile_pool(name="pagg", bufs=5, space="PSUM"))
            pp_root = ctx.enter_context(
                tc.tile_pool(name="proot", bufs=2, space="PSUM"))
            pp_tr = ctx.enter_context(
                tc.tile_pool(name="ptr", bufs=1, space="PSUM"))

            # ---- persistent SBUF
            iota_sb = cpool.tile([128, 128], BF16, tag="iota")
            nc.sync.dma_start(out=iota_sb[:], in_=iota_in[:])
            ident_raw = cpool.tile([128, 128], F32, tag="ident_raw")
            make_identity(nc, ident_raw[:])
            ident_f = cpool.tile([128, 128], F32R, tag="ident_f")
            nc.vector.tensor_copy(out=ident_f[:], in_=ident_raw[:])
            ident_b = cpool.tile([128, 128], BF16, tag="ident_b")
            nc.vector.tensor_copy(out=ident_b[:], in_=ident_raw[:])
            ones_row = cpool.tile([1, 128], BF16, tag="ones_row")
            nc.gpsimd.memset(ones_row[:], 1.0)
            binc = cpool.tile([H, 1], F32, tag="binc")
            nc.sync.dma_start(out=binc[:], in_=b_in_col[:])
            win_sb = cpool.tile([128, FB * H], F32R, tag="win")
            for fb in range(FB):
                nc.sync.dma_start(out=win_sb[:, fb * H:(fb + 1) * H],
                                  in_=win4[fb])
            initw_sb = cpool.tile([128, L * KH], BF16, tag="initw")
            for l in range(L):
                nc.sync.dma_start(out=initw_sb[:, l * KH:(l + 1) * KH],
                                  in_=initw2[l])
            rootw_sb = cpool.tile([128, L * T * KH], BF16, tag="rootw")
            for l in range(L):
                for t in range(T):
                    o = (l * T + t) * KH
                    nc.sync.dma_start(out=rootw_sb[:, o:o + KH],
                                      in_=rootw2[l, t])
            bias_sb = cpool.tile([1, L * T * KH], BF16, tag="bias")
            for l in range(L):
                for t in range(T):
                    o = (l * T + t) * KH
                    nc.sync.dma_start(out=bias_sb[:, o:o + KH], in_=bias2[l, t])
            wmat_sb = cpool.tile([128, L * max(1, T - 1) * K * H], BF16,
                                 tag="wmat")
            for l in range(L):
                for t in range(max(1, T - 1)):
                    for k in range(K):
                        o = ((l * max(1, T - 1) + t) * K + k) * H
                        nc.sync.dma_start(out=wmat_sb[:, o:o + H],
                                          in_=wmat[l, t, k])
            wout_sb = cpool.tile([H, C], BF16, tag="wout")
            nc.sync.dma_start(out=wout_sb[:], in_=wout[:])
            bout_sb = cpool.tile([1, C], BF16, tag="bout")
            nc.sync.dma_start(out=bout_sb[:], in_=bout[:])
            gidx_sb = cpool.tile([128, SLOTS // 16], I16, tag="gidx")
            nc.sync.dma_start(out=gidx_sb[:], in_=gidx[:])
            gdl_sb = cpool.tile([128, NTILES], BF16, tag="gdl")
            nc.sync.dma_start(out=gdl_sb[:], in_=gdl[:])
            gnw_sb = cpool.tile([128, NTILES], BF16, tag="gnw")
            nc.sync.dma_start(out=gnw_sb[:], in_=gnw[:])
            hT_sb = cpool.tile([128, NPAD], BF16, tag="hT")
            acc_sb = cpool.tile([128, MAXHALF * KH], BF16, tag="acc")

            _nidx_regs = {}

            def nidx_reg(v):
                if v not in _nidx_regs:
                    _nidx_regs[v] = nc.gpsimd.to_reg(v)
                return _nidx_regs[v]

            def m_bulk(tile_base, nt):
                """One-hot-times-norm tiles for a whole call in 2 DVE ops."""
                mt = mpool.tile([128, cfg.GB * TGC, 128], BF16, tag="m")
                msl = mt[:, :nt, :]
                iota_b = iota_sb[:].unsqueeze(1).broadcast_to([128, nt, 128])
                dl_b = gdl_sb[:, tile_base:tile_base + nt].unsqueeze(2) \
                    .broadcast_to([128, nt, 128])
                nw_b = gnw_sb[:, tile_base:tile_base + nt].unsqueeze(2) \
                    .broadcast_to([128, nt, 128])
                nc.vector.tensor_tensor(out=msl, in0=iota_b, in1=dl_b,
                                        op=ALU.is_equal)
                nc.vector.tensor_tensor(out=msl, in0=msl, in1=nw_b,
                                        op=ALU.mult)
                return mt

            call_info = {(g, q): (blocks, slot_base, tile_base)
                         for (g, q, blocks, slot_base, tile_base) in calls}
            gq_counter = [0]

            for _rep in range(repeat):
                def maybe_ag(b, p):
                    # launch quarter AllGather for phase p once its last
                    # block's table rows have been written
                    if p >= L * T:
                        return
                    for q in range(4):
                        if b == dv.qstart[q + 1] - 1:
                            if p % 2 == 0:
                                ins, outs = zin_q[q], ztab_q[q]
                            else:
                                ins, outs = yin_q[q], ytab_q[q]
                            if not no_collectives:
                                nc.gpsimd.collective_compute(
                                    "AllGather", ALU.bypass, replica_groups=rg,
                                    ins=[ins[:]], outs=[outs[:]])

                # ========== input: hT = relu(Win^T xT + b_in); z0 = h ======
                for g0 in range(0, NPAD, 256):
                    gs = min(256, NPAD - g0)
                    ps = pp_root.tile([128, KH], F32, tag="root")
                    for fb in range(FB):
                        xt = xpool.tile([128, 256], F32R, tag="xin")
                        nc.sync.dma_start(out=xt[:, :gs],
                                          in_=xT4[fb, :, g0:g0 + gs])
                        nc.tensor.matmul(ps[:, :gs],
                                         win_sb[:, fb * H:(fb + 1) * H],
                                         xt[:, :gs],
                                         start=(fb == 0), stop=(fb == FB - 1))
                    nc.scalar.activation(hT_sb[:, g0:g0 + gs], ps[:, :gs],
                                         ACT.Relu, bias=binc[:])
                    for b in range(g0 // 128, (g0 + gs) // 128):
                        bsl = slice(b * 128, (b + 1) * 128)
                        trp = pp_tr.tile([128, 128], BF16, tag="tr")
                        nc.tensor.transpose(trp[:], hT_sb[:, bsl], ident_b[:])
                        zt = ypool.tile([128, KH], BF16, tag="y")
                        nc.vector.tensor_copy(out=zt[:, :H], in_=trp[:])
                        q, r0 = qb_of(b)
                        nc.sync.dma_start(out=zin_q[q][r0:r0 + 128, :],
                                          in_=zt[:, :H])
                        if debug and _rep == 0:
                            nc.sync.dma_start(
                                out=dbg_h[b * 128:(b + 1) * 128, :],
                                in_=zt[:, :H])
                        maybe_ag(b, 0)

                # ================= sparse phases ===========================
                for p in range(L * T):
                    l, t = p // T, p % T
                    width = H if t == 0 else KH
                    tabs = ztab_q if t == 0 else ytab_q
                    rw0 = (l * T + t) * KH

                    def evict(b, hi):
                        jh = b - half_base[hi]
                        asl = acc_sb[:, jh * KH:jh * KH + width]
                        ps = pp_root.tile([128, KH], F32, tag="root")
                        # root (full width, starts the psum group)
                        nc.tensor.matmul(ps[:], hT_sb[:, b * 128:(b + 1) * 128],
                                         rootw_sb[:, rw0:rw0 + KH],
                                         start=True, stop=False)
                        if t == 0:
                            trp = pp_tr.tile([128, 256], BF16, tag="tr")
                            nc.tensor.transpose(trp[:, :128], asl,
                                                ident_b[:])
                            accT = epool.tile([128, 128], BF16, tag="accT")
                            nc.scalar.activation(accT[:],
                                                 trp[:, :128],
                                                 ACT.Copy)
                            nc.tensor.matmul(ps[:], accT[:],
                                             initw_sb[:, l * KH:(l + 1) * KH],
                                             start=False, stop=False)
                        else:
                            trp = pp_tr.tile([128, 256], BF16, tag="tr")
                            for k in range(K):
                                ksl = acc_sb[:, jh * KH + k * H:
                                             jh * KH + (k + 1) * H]
                                tsl = trp[:, k * H:(k + 1) * H]
                                nc.tensor.transpose(tsl, ksl,
                                                    ident_b[:])
                                accT = epool.tile([128, 128], BF16, tag="accT")
                                nc.scalar.activation(accT[:],
                                                     tsl,
                                                     ACT.Copy)
                                wo = (l * max(1, T - 1) * K + k) * H
                                nc.tensor.matmul(ps[:, k * H:(k + 1) * H],
                                                 accT[:], wmat_sb[:, wo:wo + H],
                                                 start=False, stop=False,
                                                 skip_group_check=True)
                        nc.tensor.matmul(ps[:], ones_row[:],
                                         bias_sb[:, rw0:rw0 + KH],
                                         start=False, stop=True,
                                         skip_group_check=True)
                        osb = epool.tile([128, KH], BF16, tag="osb")
                        nc.scalar.activation(osb[:], ps[:], ACT.Relu)
                        if debug and _rep == 0 and p == 0:
                            dba = epool.tile([128, H], F32, tag="dba")
                            nc.vector.tensor_copy(out=dba[:], in_=asl)
                            nc.sync.dma_start(
                                out=dbg_agg[b * 128:(b + 1) * 128, :],
                                in_=dba[:])
                            nc.sync.dma_start(
                                out=dbg_out0[b * 128:(b + 1) * 128, :],
                                in_=osb[:])
                        q, r0 = qb_of(b)
                        if t == 0:
                            # table for t=1 phase: y = out0
                            nc.sync.dma_start(out=yin_q[q][r0:r0 + 128, :],
                                              in_=osb[:])
                            maybe_ag(b, p + 1)
                        else:
                            # h_next = relu(mean_k out)
                            hs = epool.tile([128, H], BF16, tag="hs")
                            nc.vector.tensor_tensor(out=hs[:], in0=osb[:, :H],
                                                    in1=osb[:, H:KH],
                                                    op=ALU.add)
                            hr = epool.tile([128, H], BF16, tag="hr")
                            nc.scalar.activation(hr[:], hs[:], ACT.Relu,
                                                 scale=1.0 / K)
                            if p < L * T - 1:
                                nc.sync.dma_start(out=zin_q[q][r0:r0 + 128, :],
                                                  in_=hr[:])
                                maybe_ag(b, p + 1)
                            trp = pp_tr.tile([128, 128], BF16, tag="tr")
                            nc.tensor.transpose(trp[:], hr[:], ident_b[:])
                            nc.scalar.activation(
                                hT_sb[:, b * 128:(b + 1) * 128],
                                trp[:], ACT.Copy)
                            if p == L * T - 1:
                                lp = pp_root.tile([128, KH], F32, tag="root")
                                nc.tensor.matmul(
                                    lp[:, :C],
                                    hT_sb[:, b * 128:(b + 1) * 128],
                                    wout_sb[:], start=True, stop=False)
                                nc.tensor.matmul(lp[:, :C], ones_row[:],
                                                 bout_sb[:], start=False,
                                                 stop=True,
                                                 skip_group_check=True)
                                ls = epool.tile([128, C], F32, tag="ls")
                                nc.vector.tensor_copy(out=ls[:], in_=lp[:, :C])
                                nc.sync.dma_start(
                                    out=logits[b * 128:(b + 1) * 128, :],
                                    in_=ls[:])

                    cap = 4 if width == H else 2  # blocks packed per PSUM bank
                    for hi, half in enumerate(dv.halves):
                        for pair in range(2):
                            for g in half:
                                blocks = dv.groups[g]
                                psums = {}

                                def agg_slice(j):
                                    pt = psums[j // cap]
                                    return pt[:, (j % cap) * width:
                                              (j % cap + 1) * width]

                                for qi, q in enumerate((2 * pair,
                                                        2 * pair + 1)):
                                    _, slot_base, tile_base = call_info[(g, q)]
                                    nt = len(blocks) * TGC
                                    gt = gpool.tile([128, GB * TGC, width],
                                                    BF16, tag="gath")
                                    if not no_gathers:
                                        nc.gpsimd.dma_gather(
                                            gt[:, :nt, :], tabs[q][:],
                                            gidx_sb[:, slot_base // 16:
                                                    (slot_base + nt * 128) // 16],
                                            num_idxs=nt * 128,
                                            num_idxs_reg=nidx_reg(nt * 128),
                                            elem_size=width,
                                            single_packet=False,
                                            queue_num=gq_counter[0] % 4)
                                        gq_counter[0] += 1
                                    mt = m_bulk(tile_base, nt)
                                    for j, b in enumerate(blocks):
                                        if qi == 0 and j % cap == 0:
                                            psums[j // cap] = pp_agg.tile(
                                                [128, 512], F32, tag="agg",
                                                name=f"agg_{_rep}_{p}_{g}"
                                                     f"_{pair}_{j // cap}")
                                        # start=True clears has_written for
                                        # the WHOLE bank: only the first
                                        # matmul of each bank may set it.
                                        jlast = min(j // cap * cap + cap - 1,
                                                    len(blocks) - 1)
                                        for t2 in range(TGC):
                                            nc.tensor.matmul(
                                                agg_slice(j),
                                                mt[:, j * TGC + t2, :],
                                                gt[:, j * TGC + t2, :],
                                                start=(qi == 0 and t2 == 0
                                                       and j % cap == 0),
                                                stop=(qi == 1 and
                                                      t2 == TGC - 1 and
                                                      j == jlast),
                                                skip_group_check=True)
                                for j, b in enumerate(blocks):
                                    jh = b - half_base[hi]
                                    asl = acc_sb[:, jh * KH:jh * KH + width]
                                    if pair == 0:
                                        nc.vector.tensor_copy(
                                            out=asl, in_=agg_slice(j))
                                    else:
                                        nc.vector.tensor_tensor(
                                            out=asl, in0=asl,
                                            in1=agg_slice(j),
                                            op=ALU.add)
                                        evict(b, hi)

    lower_extended_insts(nc)
    return nc


_CACHE = {}


def _get_built(cfg, TGC, repeat=1, debug=False, **kw):
    key = (cfg, TGC, repeat, debug, tuple(sorted(kw.items())))
    if key not in _CACHE:
        _CACHE[key] = build_nc(cfg, derive(cfg), TGC, repeat=repeat,
                               debug=debug, **kw)
    return _CACHE[key]


def run_on_hw(cfg, inputs, trace=False, debug=False):
    from concourse.bass_utils import run_bass_kernel_spmd

    dv = derive(cfg)
    per_core, TGC = pack_host(cfg, dv, **inputs)
    nc = _get_built(cfg, TGC, repeat=1, debug=debug)
    if not getattr(nc, "_waits_fixed", False):
        fix_excess_waits(nc)
        nc._waits_fixed = True
    res = run_bass_kernel_spmd(nc, per_core, list(range(cfg.W)), trace=trace)
    out = np.concatenate(
        [res.results[c]["logits"][:dv.NPper] for c in range(cfg.W)], axis=0
    )
    return out, res


def kernel(**inputs) -> np.ndarray:
    cfg = Cfg()
    out, _ = run_on_hw(cfg, inputs)
    return out.astype(np.float32)

